# revision 1
# baseline (speedup 1.0000x reference)
"""Gemma-style sliding-window attention block on 8 trn2 NeuronCores.

Sharding: tensor-parallel over kv-head groups (4) x data-parallel over
batch (2).  Core c handles batch b = c//4 and kv-head g = c%4 (query
heads 2g, 2g+1).  Each core computes its heads' Q/K/V projections,
RMS norms, RoPE, sliding-window attention and the partial Wo
projection; the host sums the 4 partial outputs per batch.

All matmuls run in float32r (fp32 with 11-bit mantissa, full PE rate at
free-dim >= 256).  Host pre-rounds DMA'd operands; on-chip producers
write f32r directly.  Softmax is row-layout (queries on partitions)
with exact row max; attn tiles are PE-transposed for the AV matmul.
"""
import numpy as np
from contextlib import ExitStack

import concourse.bass as bass
import concourse.bacc as bacc
import concourse.mybir as mybir
import concourse.tile as tile
from concourse.bass_utils import run_bass_kernel_spmd

F32 = mybir.dt.float32
F32R = mybir.dt.float32r
AL = mybir.AluOpType
AF = mybir.ActivationFunctionType

B, S, H = 2, 2048, 2560
NH, NKV, D = 8, 4, 256
SW = 1024
EPS = 1e-6
ST = S // 128            # 16 sequence tiles
KT = H // 128            # 20 hidden k-tiles
NSC = S // 512           # 4 sequence chunks
WT = 9                   # window tiles per query tile (1024/128 + 1)
DQ = 2 * D               # per-core query dims (2 heads)
NDQ = DQ // 128          # 4
NDK = D // 128           # 2


def round_f32r(x: np.ndarray) -> np.ndarray:
    """Round fp32 to f32r (11-bit mantissa, round-to-nearest-even)."""
    b = np.ascontiguousarray(x, dtype=np.float32).view(np.uint32).astype(np.uint64)
    bias = 0x7FF + ((b >> 12) & 1)
    return ((b + bias) & 0xFFFFF000).astype(np.uint32).view(np.float32)


def build_nc(debug=False):
    nc = bacc.Bacc("TRN2", target_bir_lowering=False, debug=False)

    hsT_d = nc.dram_tensor("hsT", [KT, 128, S], F32R, kind="ExternalInput")
    wq_d = nc.dram_tensor("wqT", [KT, 128, DQ], F32R, kind="ExternalInput")
    wk_d = nc.dram_tensor("wkT", [KT, 128, D], F32R, kind="ExternalInput")
    wv_d = nc.dram_tensor("wvT", [KT, 128, D], F32R, kind="ExternalInput")
    wo_d = nc.dram_tensor("woT", [NDQ, 128, H], F32R, kind="ExternalInput")
    cos_d = nc.dram_tensor("cosT", [NDK, 128, S], F32, kind="ExternalInput")
    sin_d = nc.dram_tensor("sinT", [NDK, 128, S], F32, kind="ExternalInput")
    msk_d = nc.dram_tensor("masks", [ST, 3, 128, 384], F32R, kind="ExternalInput")
    idn_d = nc.dram_tensor("ident", [128, 128], F32R, kind="ExternalInput")
    ones_d = nc.dram_tensor("ones_c", [128, 1], F32R, kind="ExternalInput")
    onesr_d = nc.dram_tensor("onesr_c", [1, 128], F32R, kind="ExternalInput")
    qw_d = nc.dram_tensor("qw1p", [128, NDK], F32, kind="ExternalInput")
    kw_d = nc.dram_tensor("kw1p", [128, NDK], F32, kind="ExternalInput")
    out_d = nc.dram_tensor("out", [S, H], F32, kind="ExternalOutput")
    dbg = {}
    if debug:
        for nm, shp in [("dQT0", [128, S]), ("dKT0", [128, S]),
                        ("dV", [128, ST * D]), ("dexpb", [128, 1152]),
                        ("dao", [128, D]), ("daoT0", [128, S]),
                        ("dsc", [128, 1152]), ("dnegm", [128, 1])]:
            dbg[nm] = nc.dram_tensor(nm, shp, F32, kind="ExternalOutput")

    with ExitStack() as top:
        tc = top.enter_context(tile.TileContext(nc))
        big = top.enter_context(tc.tile_pool(name="big", bufs=1))

        # Resident tensors (whole-kernel lifetime)
        QT = [big.tile([128, S], F32R, name=f"QT{j}", tag=f"QT{j}") for j in range(NDQ)]
        KTt = [big.tile([128, S], F32R, name=f"KTt{j}", tag=f"KTt{j}") for j in range(NDK)]
        V = big.tile([128, ST, D], F32R, tag="V")
        aoT = [big.tile([128, S], F32R, name=f"aoT{j}", tag=f"aoT{j}") for j in range(NDQ)]
        ident = big.tile([128, 128], F32R, tag="ident")
        ones = big.tile([128, 1], F32R, tag="ones")
        onesr = big.tile([1, 128], F32R, tag="onesr")
        epsb = big.tile([128, 1], F32, tag="epsb")
        qw1p = big.tile([128, NDK], F32, tag="qw1p")
        kw1p = big.tile([128, NDK], F32, tag="kw1p")
        nc.sync.dma_start(out=ident, in_=idn_d[:, :])
        nc.sync.dma_start(out=qw1p, in_=qw_d[:, :])
        nc.sync.dma_start(out=kw1p, in_=kw_d[:, :])
        nc.sync.dma_start(out=ones, in_=ones_d[:, :])
        nc.sync.dma_start(out=onesr, in_=onesr_d[:, :])
        nc.vector.memset(epsb, EPS)

        # ---------------- Phase 1: projections + norms + rope -------------
        with ExitStack() as p1:
            wpool = p1.enter_context(tc.tile_pool(name="wpool", bufs=1))
            wstr = p1.enter_context(tc.tile_pool(name="wstr", bufs=3))
            hpool = p1.enter_context(tc.tile_pool(name="hpool", bufs=3))
            cpool = p1.enter_context(tc.tile_pool(name="cpool", bufs=2))
            tpool = p1.enter_context(tc.tile_pool(name="tpool", bufs=1))
            spool = p1.enter_context(tc.tile_pool(name="spool", bufs=2))
            spool1 = p1.enter_context(tc.tile_pool(name="spool1", bufs=1))
            pps = p1.enter_context(tc.tile_pool(name="pps", bufs=1, space="PSUM"))

            wq = wpool.tile([128, KT, DQ], F32R, tag="wq")
            wk = wpool.tile([128, KT, D], F32R, tag="wk")
            nc.sync.dma_start(out=wq, in_=wq_d.rearrange("k p m -> p k m"))
            nc.sync.dma_start(out=wk, in_=wk_d.rearrange("k p m -> p k m"))

            for sc in range(NSC):
                sl = slice(sc * 512, (sc + 1) * 512)
                qps = [pps.tile([128, 512], F32, name=f"qps{j}", tag=f"qps{j}") for j in range(NDQ)]
                kps = [pps.tile([128, 512], F32, name=f"kps{j}", tag=f"kps{j}") for j in range(NDK)]
                vps = pps.tile([128, 4, D], F32, tag="vps")
                vflat = vps.rearrange("p a b -> p (a b)")

                for kt in range(KT):
                    hst = hpool.tile([128, 512], F32R, tag="hst")
                    nc.sync.dma_start(out=hst, in_=hsT_d[kt, :, sl])
                    wv = wstr.tile([128, D], F32R, tag="wv")
                    nc.sync.dma_start(out=wv, in_=wv_d[kt, :, :])
                    st_, sp_ = (kt == 0), (kt == KT - 1)
                    for j in range(NDQ):
                        nc.tensor.matmul(qps[j], wq[:, kt, j * 128:(j + 1) * 128],
                                         hst, start=st_, stop=sp_)
                    for j in range(NDK):
                        nc.tensor.matmul(kps[j], wk[:, kt, j * 128:(j + 1) * 128],
                                         hst, start=st_, stop=sp_)
                    for i in range(4):
                        # i in {1,3} shares a PSUM bank with i-1; start=True
                        # clears the whole bank, so only the first sub-tile
                        # per bank starts the group (has_written bits make the
                        # sibling's first write an overwrite).
                        nc.tensor.matmul(vps[:, i, :], hst[:, i * 128:(i + 1) * 128],
                                         wv, start=(st_ and i % 2 == 0), stop=sp_)

                # V rms norm (no weight): rows are sequence positions
                for i in range(4):
                    vscr = tpool.tile([128, D], F32, tag="vscr")
                    msq = spool.tile([128, 1], F32, tag="msq")
                    nc.scalar.activation(out=vscr, in_=vps[:, i, :],
                                         func=AF.Square, accum_out=msq)
                    sdv = spool.tile([128, 1], F32, tag="sdv")
                    nc.scalar.activation(out=sdv, in_=msq, func=AF.Sqrt,
                                         scale=1.0 / D, bias=epsb)
                    rv = spool.tile([128, 1], F32, tag="rv")
                    nc.vector.reciprocal(out=rv, in_=sdv)
                    nc.vector.tensor_scalar_mul(V[:, sc * 4 + i, :], vps[:, i, :], rv)

                # Q/K rms norm + rope (transposed layout: d on partitions)
                # heads: (dst tiles, psum tiles, d-tile idx pairs, weight)
                heads = [(QT, qps, (0, 1), qw1p), (QT, qps, (2, 3), qw1p),
                         (KTt, kps, (0, 1), kw1p)]
                cosA = cpool.tile([128, 512], F32, tag="cosA")
                cosB = cpool.tile([128, 512], F32, tag="cosB")
                sinA = cpool.tile([128, 512], F32, tag="sinA")
                sinB = cpool.tile([128, 512], F32, tag="sinB")
                nc.sync.dma_start(out=cosA, in_=cos_d[0, :, sl])
                nc.sync.dma_start(out=cosB, in_=cos_d[1, :, sl])
                nc.sync.dma_start(out=sinA, in_=sin_d[0, :, sl])
                nc.sync.dma_start(out=sinB, in_=sin_d[1, :, sl])
                for hidx, (dst, src, (jA, jB), w1p) in enumerate(heads):
                    ssq_home = vflat[0:1, 0:512] if hidx != 1 else vflat[0:1, 512:1024]
                    sq = [tpool.tile([128, 512], F32R, name=f"sq{j}", tag=f"sq{j}") for j in (0, 1)]
                    for j, jj in enumerate((jA, jB)):
                        nc.scalar.activation(out=sq[j], in_=src[jj], func=AF.Square)
                    nc.tensor.matmul(ssq_home, ones, sq[0], start=True, stop=False)
                    nc.tensor.matmul(ssq_home, ones, sq[1], start=False, stop=True)
                    sd = spool1.tile([1, 512], F32, tag="sd")
                    nc.scalar.activation(out=sd, in_=ssq_home, func=AF.Sqrt,
                                         scale=1.0 / D, bias=epsb[0:1, :])
                    rqf = spool1.tile([1, 512], F32, tag="rqf")
                    nc.vector.reciprocal(out=rqf, in_=sd)
                    # hi/lo split so the f32r rank-1 broadcast is fp32-exact
                    rq = spool1.tile([1, 512], F32R, tag="rq")
                    nc.vector.tensor_copy(out=rq, in_=rqf)
                    rql = spool1.tile([1, 512], F32R, tag="rql")
                    with nc.allow_low_precision(reason="f32r lo residual"):
                        nc.vector.tensor_sub(rql, rqf, rq)
                    bcps = vflat[:, 0:512] if hidx != 1 else vflat[:, 512:1024]
                    nc.tensor.matmul(bcps, onesr, rq, start=True, stop=False)
                    nc.tensor.matmul(bcps, onesr, rql, start=False, stop=True)
                    bc = tpool.tile([128, 512], F32, tag="bc")
                    nc.scalar.copy(out=bc, in_=bcps)
                    qn = []
                    for j, jj in enumerate((jA, jB)):
                        q = tpool.tile([128, 512], F32, name=f"qn{j}", tag=f"qn{j}")
                        nc.vector.scalar_tensor_tensor(
                            out=q, in0=src[jj], scalar=w1p[:, j:j + 1],
                            in1=bc, op0=AL.mult, op1=AL.mult)
                        qn.append(q)
                    t1 = tpool.tile([128, 512], F32, tag="t1")
                    t2 = tpool.tile([128, 512], F32, tag="t2")
                    nc.vector.tensor_mul(t1, qn[0], cosA)
                    nc.vector.tensor_mul(t2, qn[1], sinA)
                    nc.vector.tensor_sub(dst[jA][:, sl], t1, t2)
                    t3 = tpool.tile([128, 512], F32, tag="t1")
                    t4 = tpool.tile([128, 512], F32, tag="t2")
                    nc.vector.tensor_mul(t3, qn[1], cosB)
                    nc.vector.tensor_mul(t4, qn[0], sinB)
                    nc.vector.tensor_add(dst[jB][:, sl], t3, t4)

        if debug:
            nc.sync.dma_start(out=dbg["dQT0"][:, :], in_=QT[0].bitcast(F32))
            nc.sync.dma_start(out=dbg["dKT0"][:, :], in_=KTt[0].bitcast(F32))
            nc.sync.dma_start(out=dbg["dV"][:, :],
                              in_=V.rearrange("p a b -> p (a b)").bitcast(F32))

        # ---------------- Phase 2: attention ------------------------------
        with ExitStack() as p23:
            wopool = p23.enter_context(tc.tile_pool(name="wopool", bufs=1))
            p2 = p23.enter_context(ExitStack())
            mpool = p2.enter_context(tc.tile_pool(name="mpool", bufs=2))
            epool = p2.enter_context(tc.tile_pool(name="epool", bufs=3))
            npool = p2.enter_context(tc.tile_pool(name="npool", bufs=3))
            scps = p2.enter_context(tc.tile_pool(name="scps", bufs=1, space="PSUM"))
            trps = p2.enter_context(tc.tile_pool(name="trps", bufs=2, space="PSUM"))
            aops_p = p2.enter_context(tc.tile_pool(name="aops", bufs=3, space="PSUM"))

            woT = wopool.tile([128, NDQ, H], F32R, tag="woT")
            nc.sync.dma_start(out=woT, in_=wo_d.rearrange("k p m -> p k m"))

            dbg_sc_sb = (epool.tile([128, 1152], F32, name="dbgsc", tag="dbgsc")
                         if debug else None)
            for t in range(ST):
                w0 = max(0, t - 8)
                msk = mpool.tile([128, 3, 384], F32R, tag="msk")
                nc.sync.dma_start(out=msk,
                                  in_=msk_d[t].rearrange("c p n -> p c n"))
                mask_chunks = (0, 1, 2) if t < 8 else (0, 2)
                for h in range(2):
                    scs = [scps.tile([128, 512], F32, name=f"sc{c}", tag=f"sc{c}")[:, :384]
                           for c in range(3)]
                    for c in range(3):
                        has_mask = c in mask_chunks
                        rhs_sl = slice(w0 * 128 + c * 384, w0 * 128 + c * 384 + 384)
                        for j in range(NDK):
                            nc.tensor.matmul(
                                scs[c], QT[2 * h + j][:, t * 128:(t + 1) * 128],
                                KTt[j][:, rhs_sl], start=(j == 0),
                                stop=(j == 1 and not has_mask))
                        if has_mask:
                            nc.tensor.matmul(scs[c], ident, msk[:, c, :],
                                             start=False, stop=True)
                    nm = [npool.tile([128, 1], F32, name=f"nm{c}", tag=f"nm{c}") for c in range(3)]
                    for c in range(3):
                        nc.vector.tensor_reduce(out=nm[c], in_=scs[c],
                                                axis=mybir.AxisListType.X,
                                                op=AL.max, negate=True)
                    negm = npool.tile([128, 1], F32, tag="negm")
                    nc.vector.tensor_tensor(negm, nm[0], nm[1], op=AL.min)
                    nc.vector.tensor_tensor(negm, negm, nm[2], op=AL.min)
                    expb = epool.tile([128, 1152], F32R, tag="expb")
                    den = npool.tile([128, 3], F32, tag="den")
                    for c in range(3):
                        nc.scalar.activation(out=expb[:, c * 384:(c + 1) * 384],
                                             in_=scs[c], func=AF.Exp, bias=negm,
                                             accum_out=den[:, c:c + 1])
                    dsum = npool.tile([128, 1], F32, tag="dsum")
                    nc.vector.tensor_reduce(out=dsum, in_=den,
                                            axis=mybir.AxisListType.X, op=AL.add)
                    rden = npool.tile([128, 1], F32, tag="rden")
                    nc.vector.reciprocal(out=rden, in_=dsum)

                    if debug and t == 10 and h == 0:
                        nc.sync.dma_start(out=dbg["dexpb"][:, :],
                                          in_=expb.bitcast(F32))
                        nc.sync.dma_start(out=dbg["dnegm"][:, :], in_=negm)
                        for c in range(3):
                            nc.vector.tensor_copy(
                                out=dbg_sc_sb[:, c * 384:(c + 1) * 384],
                                in_=scs[c])
                        nc.sync.dma_start(out=dbg["dsc"][:, :], in_=dbg_sc_sb)
                    expT = epool.tile([128, WT, 128], F32R, tag="expT")
                    for a in range(WT):
                        trp = trps.tile([128, 128], F32R, tag="trp")
                        nc.tensor.transpose(trp, expb[:, a * 128:(a + 1) * 128],
                                            ident)
                        if a % 2 == 0:
                            nc.vector.tensor_copy(out=expT[:, a, :], in_=trp)
                        else:
                            nc.scalar.copy(out=expT[:, a, :], in_=trp)
                    aop = aops_p.tile([128, D], F32, tag="aop")
                    for a in range(WT):
                        nc.tensor.matmul(aop, expT[:, a, :], V[:, w0 + a, :],
                                         start=(a == 0), stop=(a == WT - 1))
                    ao = epool.tile([128, D], F32R, tag="ao")
                    nc.scalar.activation(out=ao, in_=aop, func=AF.Copy, scale=rden)
                    if debug and t == 10 and h == 0:
                        nc.sync.dma_start(out=dbg["dao"][:, :],
                                          in_=ao.bitcast(F32))
                    for j in range(2):
                        trp = trps.tile([128, 128], F32R, tag="trp")
                        nc.tensor.transpose(trp, ao[:, j * 128:(j + 1) * 128], ident)
                        nc.vector.tensor_copy(
                            out=aoT[2 * h + j][:, t * 128:(t + 1) * 128], in_=trp)

            if debug:
                nc.sync.dma_start(out=dbg["daoT0"][:, :], in_=aoT[0].bitcast(F32))
            # ------------- Phase 3: output projection ---------------------
            p2.close()
            with ExitStack() as p3:
                opool = p3.enter_context(tc.tile_pool(name="opool", bufs=3))
                wops = p3.enter_context(tc.tile_pool(name="wops", bufs=2,
                                                     space="PSUM"))
                for st_i in range(ST):
                    for hc in range(H // 512):
                        wop = wops.tile([128, 512], F32, tag="wop")
                        for dj in range(NDQ):
                            nc.tensor.matmul(
                                wop, aoT[dj][:, st_i * 128:(st_i + 1) * 128],
                                woT[:, dj, hc * 512:(hc + 1) * 512],
                                start=(dj == 0), stop=(dj == NDQ - 1))
                        osb = opool.tile([128, 512], F32, tag="osb")
                        if (st_i + hc) % 2 == 0:
                            nc.vector.tensor_copy(out=osb, in_=wop)
                        else:
                            nc.scalar.copy(out=osb, in_=wop)
                        nc.sync.dma_start(
                            out=out_d[st_i * 128:(st_i + 1) * 128,
                                      hc * 512:(hc + 1) * 512], in_=osb)

    nc.compile()
    return nc


_nc_cache = None


def kernel(hidden_states, attention_mask, cos, sin, Wq, Wk, Wv, Wo,
           q_norm_w, k_norm_w):
    global _nc_cache
    if _nc_cache is None:
        _nc_cache = build_nc()
    nc = _nc_cache

    hidden_states = np.asarray(hidden_states, dtype=np.float32)
    mask = np.asarray(attention_mask, dtype=np.float32)[0, 0]      # [S, S]
    cos2 = np.asarray(cos, dtype=np.float32)[0, 0]                 # [S, D]
    sin2 = np.asarray(sin, dtype=np.float32)[0, 0]
    Wq = np.asarray(Wq, dtype=np.float32)
    Wk = np.asarray(Wk, dtype=np.float32)
    Wv = np.asarray(Wv, dtype=np.float32)
    Wo = np.asarray(Wo, dtype=np.float32)

    cosT = np.ascontiguousarray(cos2.T.reshape(NDK, 128, S))
    sinT = np.ascontiguousarray(sin2.T.reshape(NDK, 128, S))

    # Row-layout mask tiles: for query tile t, key chunks of 384 starting
    # at 128*max(0, t-8).
    masks = np.zeros((ST, 3, 128, 384), dtype=np.float32)
    for t in range(ST):
        w0 = max(0, t - 8)
        rows = slice(t * 128, (t + 1) * 128)
        for c in range(3):
            cols = slice(w0 * 128 + c * 384, w0 * 128 + c * 384 + 384)
            masks[t, c] = mask[rows, cols]
    masks = round_f32r(masks)
    ident = round_f32r(np.eye(128, dtype=np.float32))

    in_maps = []
    for core in range(8):
        b, g = core // 4, core % 4
        hsT = round_f32r(np.ascontiguousarray(
            hidden_states[b].T).reshape(KT, 128, S))
        wqT = round_f32r(np.ascontiguousarray(
            Wq[2 * g * D:(2 * g + 2) * D].T).reshape(KT, 128, DQ))
        wkT = round_f32r(np.ascontiguousarray(
            Wk[g * D:(g + 1) * D].T).reshape(KT, 128, D))
        wvT = round_f32r(np.ascontiguousarray(
            Wv[g * D:(g + 1) * D].T).reshape(KT, 128, D))
        woT = round_f32r(np.ascontiguousarray(
            Wo[:, 2 * g * D:(2 * g + 2) * D].T).reshape(NDQ, 128, H))
        qw1p = np.ascontiguousarray(
            (1.0 + np.asarray(q_norm_w, dtype=np.float32)).reshape(NDK, 128).T)
        kw1p = np.ascontiguousarray(
            (1.0 + np.asarray(k_norm_w, dtype=np.float32)).reshape(NDK, 128).T)
        in_maps.append({
            "hsT": hsT, "wqT": wqT, "wkT": wkT, "wvT": wvT, "woT": woT,
            "cosT": cosT, "sinT": sinT, "masks": masks, "ident": ident,
            "ones_c": np.ones((128, 1), dtype=np.float32),
            "onesr_c": np.ones((1, 128), dtype=np.float32),
            "qw1p": qw1p, "kw1p": kw1p,
        })

    res = run_bass_kernel_spmd(nc, in_maps, core_ids=list(range(8)))
    outs = [r["out"] for r in res.results]
    final = np.zeros((B, S, H), dtype=np.float32)
    for core in range(8):
        b = core // 4
        final[b] += outs[core]
    return final



# revision 7
# speedup vs baseline: 1.3009x; 1.3009x over previous
"""Gemma-style sliding-window attention block on 8 trn2 NeuronCores.

Sharding: tensor-parallel over kv-head groups (4) x data-parallel over
batch (2).  Core c handles batch b = c//4 and kv-head g = c%4 (query
heads 2g, 2g+1).  Each core computes its heads' Q/K/V projections,
RMS norms, RoPE, sliding-window attention and the partial Wo
projection; the host sums the 4 partial outputs per batch.

All matmuls run in float32r (fp32 with 11-bit mantissa, full PE rate at
free-dim >= 256).  Host pre-rounds DMA'd operands; on-chip producers
write f32r directly.

Schedule notes (v2): projections run in 256-seq chunks so the Q/K/V
PSUM banks double-buffer (4 banks/chunk); sum-of-squares and the 1/rms
rank-1 broadcasts use a dedicated single-matmul PSUM stat bank so no
accumulation group shares a bank with a foreign start=True.  Attention
and the output projection are merged per query tile: scores for the
9-tile sliding window land in one [128,3,512] PSUM tile (bufs=2), one
strided exp covers all chunks, attn-weight transposes go 4-at-a-time
through two rotating PSUM banks that double as the Wo accumulators,
and fully-masked chunks of early query tiles are skipped outright.
Masking uses 4 resident additive patterns instead of per-tile DMA.
"""
import numpy as np
from contextlib import ExitStack

import concourse.bass as bass
import concourse.bacc as bacc
import concourse.mybir as mybir
import concourse.tile as tile
from concourse.bass_utils import run_bass_kernel_spmd

F32 = mybir.dt.float32
F32R = mybir.dt.float32r
AL = mybir.AluOpType
AF = mybir.ActivationFunctionType

B, S, H = 2, 2048, 2560
NH, NKV, D = 8, 4, 256
SW = 1024
EPS = 1e-6
ST = S // 128             # 16 sequence tiles
KT = H // 128             # 20 hidden k-tiles
NC_CH = S // 256          # 8 projection chunks of 256
DQ = 2 * D                # per-core query dims (2 heads)
NDQ = DQ // 128           # 4
NDK = D // 128            # 2


def round_f32r(x: np.ndarray) -> np.ndarray:
    """Round fp32 to f32r (11-bit mantissa, round-to-nearest-even)."""
    b = np.ascontiguousarray(x, dtype=np.float32).view(np.uint32).astype(np.uint64)
    bias = 0x7FF + ((b >> 12) & 1)
    return ((b + bias) & 0xFFFFF000).astype(np.uint32).view(np.float32)


def build_nc(debug=False):
    nc = bacc.Bacc("TRN2", target_bir_lowering=False, debug=False)

    hsT_d = nc.dram_tensor("hsT", [KT, 128, S], F32R, kind="ExternalInput")
    wq_d = nc.dram_tensor("wqT", [KT, 128, DQ], F32R, kind="ExternalInput")
    wk_d = nc.dram_tensor("wkT", [KT, 128, D], F32R, kind="ExternalInput")
    wv_d = nc.dram_tensor("wvT", [KT, 128, D], F32R, kind="ExternalInput")
    wo_d = nc.dram_tensor("woT", [NDQ, 128, H], F32R, kind="ExternalInput")
    cos_d = nc.dram_tensor("cosT", [128, S], F32, kind="ExternalInput")
    sin_d = nc.dram_tensor("sinT", [128, S], F32, kind="ExternalInput")
    msk_d = nc.dram_tensor("masks", [4, 128, 384], F32R, kind="ExternalInput")
    idn_d = nc.dram_tensor("ident", [128, 128], F32R, kind="ExternalInput")
    ones_d = nc.dram_tensor("ones_c", [128, 1], F32R, kind="ExternalInput")
    onesr_d = nc.dram_tensor("onesr_c", [1, 128], F32R, kind="ExternalInput")
    qw_d = nc.dram_tensor("qw1p", [128, NDK], F32, kind="ExternalInput")
    kw_d = nc.dram_tensor("kw1p", [128, NDK], F32, kind="ExternalInput")
    out_d = nc.dram_tensor("out", [S, H], F32, kind="ExternalOutput")

    with ExitStack() as top:
        tc = top.enter_context(tile.TileContext(nc))
        big = top.enter_context(tc.tile_pool(name="big", bufs=1))

        # Resident tensors (whole-kernel lifetime)
        QT = [big.tile([128, S], F32R, name=f"QT{j}", tag=f"QT{j}") for j in range(NDQ)]
        KTt = [big.tile([128, S], F32R, name=f"KTt{j}", tag=f"KTt{j}") for j in range(NDK)]
        V = big.tile([128, ST, D], F32R, tag="V")
        masks = big.tile([128, 4, 384], F32R, tag="masks")
        ident = big.tile([128, 128], F32R, tag="ident")
        ones = big.tile([128, 1], F32R, tag="ones")
        onesr = big.tile([1, 128], F32R, tag="onesr")
        epsb = big.tile([128, 1], F32, tag="epsb")
        qw1p = big.tile([128, NDK], F32, tag="qw1p")
        kw1p = big.tile([128, NDK], F32, tag="kw1p")
        nc.sync.dma_start(out=ident, in_=idn_d[:, :])
        nc.sync.dma_start(out=masks, in_=msk_d.rearrange("c p n -> p c n"))
        nc.sync.dma_start(out=qw1p, in_=qw_d[:, :])
        nc.sync.dma_start(out=kw1p, in_=kw_d[:, :])
        nc.sync.dma_start(out=ones, in_=ones_d[:, :])
        nc.sync.dma_start(out=onesr, in_=onesr_d[:, :])
        nc.vector.memset(epsb, EPS)

        # ---------------- Phase 1: projections + norms + rope -------------
        with ExitStack() as p1:
            wpool = p1.enter_context(tc.tile_pool(name="wpool", bufs=1))
            hpool = p1.enter_context(tc.tile_pool(name="hpool", bufs=5))
            cpool = p1.enter_context(tc.tile_pool(name="cpool", bufs=2))
            sqpool = p1.enter_context(tc.tile_pool(name="sqpool", bufs=2))
            tpool = p1.enter_context(tc.tile_pool(name="tpool", bufs=2))
            spool = p1.enter_context(tc.tile_pool(name="spool", bufs=2))
            pps = p1.enter_context(tc.tile_pool(name="pps", bufs=2, space="PSUM"))
            vpp = p1.enter_context(tc.tile_pool(name="vpp", bufs=1, space="PSUM"))
            stps = p1.enter_context(tc.tile_pool(name="stps", bufs=1, space="PSUM"))

            wq = wpool.tile([128, KT, DQ], F32R, tag="wq")
            wk = wpool.tile([128, KT, D], F32R, tag="wk")
            wv = wpool.tile([128, KT, D], F32R, tag="wv")
            for kt in range(KT):
                nc.sync.dma_start(out=wq[:, kt, :], in_=wq_d[kt])
                nc.sync.dma_start(out=wk[:, kt, :], in_=wk_d[kt])
                nc.sync.dma_start(out=wv[:, kt, :], in_=wv_d[kt])

            for sc in range(NC_CH):
                sl = slice(sc * 256, (sc + 1) * 256)
                hst = [hpool.tile([128, 5, 256], F32R, name=f"hst{g}", tag="hst")
                       for g in range(4)]
                for g in range(4):
                    nc.sync.dma_start(
                        out=hst[g],
                        in_=hsT_d[g * 5:(g + 1) * 5, :, sl].rearrange("k p s -> p k s"))
                cosC = cpool.tile([128, 256], F32, tag="cosC")
                sinC = cpool.tile([128, 256], F32, tag="sinC")
                nc.sync.dma_start(out=cosC, in_=cos_d[:, sl])
                nc.sync.dma_start(out=sinC, in_=sin_d[:, sl])

                qps = pps.tile([128, NDQ, 256], F32, tag="qps")
                kps = pps.tile([128, NDK, 256], F32, tag="kps")
                vps = vpp.tile([128, 2, 256], F32, tag="vps")
                stat = stps.tile([128, 512], F32, tag="stat")

                for kt in range(KT):
                    h = hst[kt // 5][:, kt % 5, :]
                    st_, sp_ = (kt == 0), (kt == KT - 1)
                    for j in range(NDQ):
                        nc.tensor.matmul(qps[:, j, :], wq[:, kt, j * 128:(j + 1) * 128],
                                         h, start=(st_ and j % 2 == 0), stop=sp_)
                    for j in range(NDK):
                        nc.tensor.matmul(kps[:, j, :], wk[:, kt, j * 128:(j + 1) * 128],
                                         h, start=(st_ and j == 0), stop=sp_)
                    for i in range(2):
                        nc.tensor.matmul(vps[:, i, :], h[:, i * 128:(i + 1) * 128],
                                         wv[:, kt, :], start=(st_ and i == 0), stop=sp_)

                # V rms norm (no weight): rows are sequence positions
                for i in range(2):
                    vscr = tpool.tile([128, 256], F32, tag="vscr")
                    msq = spool.tile([128, 1], F32, tag="msq")
                    nc.scalar.activation(out=vscr, in_=vps[:, i, :],
                                         func=AF.Square, accum_out=msq)
                    sdv = spool.tile([128, 1], F32, tag="sdv")
                    nc.scalar.activation(out=sdv, in_=msq, func=AF.Sqrt,
                                         scale=1.0 / D, bias=epsb)
                    rv = spool.tile([128, 1], F32, tag="rv")
                    nc.vector.reciprocal(out=rv, in_=sdv)
                    nc.vector.tensor_scalar_mul(V[:, sc * 2 + i, :], vps[:, i, :], rv)

                # Q/K rms norm + rope (transposed layout: d on partitions).
                # All stat-bank matmuls are single start/stop groups so a
                # foreign start=True never splits an accumulation pair.
                heads = [(QT, qps, (0, 1), qw1p), (QT, qps, (2, 3), qw1p),
                         (KTt, kps, (0, 1), kw1p)]
                for hidx, (dst, src, (jA, jB), w1p) in enumerate(heads):
                    sq = sqpool.tile([128, 2, 256], F32R, tag="sq")
                    nc.scalar.activation(out=sq, in_=src[:, jA:jA + 2, :],
                                         func=AF.Square)
                    # Sum of squares over both d-tiles: accumulation pair in
                    # the stat bank.  Region overlaps (row 0) serialize each
                    # head's chain, so no foreign start=True can split a pair.
                    ssqw = stat[0:1, 0:256]
                    nc.tensor.matmul(ssqw, ones, sq[:, 0, :], start=True, stop=False)
                    nc.tensor.matmul(ssqw, ones, sq[:, 1, :], start=False, stop=True)
                    sd = spool.tile([1, 256], F32, tag="sd")
                    nc.scalar.activation(out=sd, in_=ssqw, func=AF.Sqrt,
                                         scale=1.0 / D, bias=epsb[0:1, :])
                    rqf = spool.tile([1, 256], F32, tag="rqf")
                    nc.vector.reciprocal(out=rqf, in_=sd)
                    # hi/lo split so the f32r rank-1 broadcast is fp32-exact
                    rq = spool.tile([1, 256], F32R, tag="rq")
                    nc.vector.tensor_copy(out=rq, in_=rqf)
                    rql = spool.tile([1, 256], F32R, tag="rql")
                    with nc.allow_low_precision(reason="f32r lo residual"):
                        nc.vector.tensor_sub(rql, rqf, rq)
                    # bc region contains ssqw's row 0, so the next head's ssq
                    # pair WAR-waits on this head's bc copy: pairs never split.
                    bcps = stat[:, 0:256]
                    nc.tensor.matmul(bcps, onesr, rq, start=True, stop=False)
                    nc.tensor.matmul(bcps, onesr, rql, start=False, stop=True)
                    bc = tpool.tile([128, 256], F32, tag="bc")
                    nc.scalar.copy(out=bc, in_=bcps)
                    qn = []
                    for j, jj in enumerate((jA, jB)):
                        q = tpool.tile([128, 256], F32, name=f"qn{j}", tag=f"qn{j}")
                        nc.vector.scalar_tensor_tensor(
                            out=q, in0=src[:, jj, :], scalar=w1p[:, j:j + 1],
                            in1=bc, op0=AL.mult, op1=AL.mult)
                        qn.append(q)
                    t1 = tpool.tile([128, 256], F32, tag="t1")
                    t2 = tpool.tile([128, 256], F32, tag="t2")
                    nc.vector.tensor_mul(t1, qn[0], cosC)
                    nc.vector.tensor_mul(t2, qn[1], sinC)
                    nc.vector.tensor_sub(dst[jA][:, sl], t1, t2)
                    t3 = tpool.tile([128, 256], F32, tag="t1")
                    t4 = tpool.tile([128, 256], F32, tag="t2")
                    nc.vector.tensor_mul(t3, qn[1], cosC)
                    nc.vector.tensor_mul(t4, qn[0], sinC)
                    nc.vector.tensor_add(dst[jB][:, sl], t3, t4)

        # ------- Phase 2+3: attention + output projection, per tile -------
        with ExitStack() as p23:
            wopool = p23.enter_context(tc.tile_pool(name="wopool", bufs=1))
            scp = p23.enter_context(tc.tile_pool(name="scp", bufs=2, space="PSUM"))
            trp_p = p23.enter_context(tc.tile_pool(name="trp_p", bufs=2, space="PSUM"))
            epool = p23.enter_context(tc.tile_pool(name="epool", bufs=2))
            etp = p23.enter_context(tc.tile_pool(name="etp", bufs=2))
            npool = p23.enter_context(tc.tile_pool(name="npool", bufs=4))
            aopool = p23.enter_context(tc.tile_pool(name="aopool", bufs=2))
            opool = p23.enter_context(tc.tile_pool(name="opool", bufs=2))

            woT = wopool.tile([128, NDQ, H], F32R, tag="woT")
            for dj in range(NDQ):
                nc.sync.dma_start(out=woT[:, dj, :], in_=wo_d[dj])

            cp_i = 0  # round-robin PSUM->SBUF copies between DVE and ACT

            def psum_copy(out, in_):
                nonlocal cp_i
                if cp_i % 2 == 0:
                    nc.vector.tensor_copy(out=out, in_=in_)
                else:
                    nc.scalar.copy(out=out, in_=in_)
                cp_i += 1

            for t in range(ST):
                w0 = max(0, t - 8)
                nt = min(t + 1, 9)              # live window tiles
                nch = (nt * 128 + 383) // 384   # live 384-wide score chunks
                aot = aopool.tile([128, NDQ, 128], F32R, tag="aot")
                aot_f = aot.rearrange("p a x -> p (a x)")
                for h in range(2):
                    scs = scp.tile([128, 3, 512], F32, tag="scs")
                    for c in range(nch):
                        rhs_sl = slice(w0 * 128 + c * 384, w0 * 128 + c * 384 + 384)
                        # mask pattern for this chunk (or None)
                        if t < 8:
                            pat = t % 3 if c == t // 3 else None
                        else:
                            pat = 3 if c == 0 else (2 if c == 2 else None)
                        for j in range(NDK):
                            nc.tensor.matmul(
                                scs[:, c, 0:384], QT[2 * h + j][:, t * 128:(t + 1) * 128],
                                KTt[j][:, rhs_sl], start=(j == 0),
                                stop=(j == NDK - 1 and pat is None))
                        if pat is not None:
                            nc.tensor.matmul(scs[:, c, 0:384], ident, masks[:, pat, :],
                                             start=False, stop=True)
                    negm = npool.tile([128, 1], F32, tag="negm")
                    nc.vector.tensor_reduce(out=negm, in_=scs[:, 0:nch, 0:384],
                                            axis=mybir.AxisListType.XY,
                                            op=AL.max, negate=True)
                    expb = epool.tile([128, 3, 384], F32R, tag="expb")
                    expb_f = expb.rearrange("p c x -> p (c x)")
                    dsum = npool.tile([128, 1], F32, tag="dsum")
                    nc.scalar.activation(out=expb[:, 0:nch, :], in_=scs[:, 0:nch, 0:384],
                                         func=AF.Exp, bias=negm, accum_out=dsum)
                    rden = npool.tile([128, 1], F32, tag="rden")
                    nc.vector.reciprocal(out=rden, in_=dsum)
                    expT = etp.tile([128, 9, 128], F32R, tag="expT")
                    expT_f = expT.rearrange("p a x -> p (a x)")
                    for g in range((nt + 3) // 4):
                        trp = trp_p.tile([128, 512], F32R, tag="trp")
                        n_in = min(4, nt - 4 * g)
                        for a2 in range(n_in):
                            a = 4 * g + a2
                            nc.tensor.transpose(
                                trp[:, a2 * 128:(a2 + 1) * 128],
                                expb_f[:, a * 128:(a + 1) * 128], ident)
                        psum_copy(expT_f[:, 4 * g * 128:(4 * g + n_in) * 128],
                                  trp[:, 0:n_in * 128])
                    aop = scs[:, 2, 0:256]
                    for a in range(nt):
                        nc.tensor.matmul(aop, expT[:, a, :], V[:, w0 + a, :],
                                         start=(a == 0), stop=(a == nt - 1))
                    ao = epool.tile([128, 256], F32R, tag="ao")
                    nc.scalar.activation(out=ao, in_=aop, func=AF.Copy, scale=rden)
                    trp2 = trp_p.tile([128, 512], F32R, tag="trp")
                    for j in range(2):
                        nc.tensor.transpose(trp2[:, j * 128:(j + 1) * 128],
                                            ao[:, j * 128:(j + 1) * 128], ident)
                    psum_copy(aot_f[:, 2 * h * 128:(2 * h + 2) * 128],
                              trp2[:, 0:256])

                # ---- output projection for this query tile ----
                osb = opool.tile([128, H], F32, tag="osb")
                for hc in range(H // 512):
                    wop = trp_p.tile([128, 512], F32, name="wop", tag="trp")
                    for dj in range(NDQ):
                        nc.tensor.matmul(
                            wop, aot[:, dj, :], woT[:, dj, hc * 512:(hc + 1) * 512],
                            start=(dj == 0), stop=(dj == NDQ - 1))
                    psum_copy(osb[:, hc * 512:(hc + 1) * 512], wop)
                nc.sync.dma_start(
                    out=out_d[t * 128:(t + 1) * 128, :], in_=osb)

    nc.compile()
    return nc


_nc_cache = None


def kernel(hidden_states, attention_mask, cos, sin, Wq, Wk, Wv, Wo,
           q_norm_w, k_norm_w):
    global _nc_cache
    if _nc_cache is None:
        _nc_cache = build_nc()
    nc = _nc_cache

    hidden_states = np.asarray(hidden_states, dtype=np.float32)
    mask = np.asarray(attention_mask, dtype=np.float32)[0, 0]      # [S, S]
    cos2 = np.asarray(cos, dtype=np.float32)[0, 0]                 # [S, D]
    sin2 = np.asarray(sin, dtype=np.float32)[0, 0]
    Wq = np.asarray(Wq, dtype=np.float32)
    Wk = np.asarray(Wk, dtype=np.float32)
    Wv = np.asarray(Wv, dtype=np.float32)
    Wo = np.asarray(Wo, dtype=np.float32)

    # cos/sin have duplicated half-frequencies: only the first 128 rows of
    # the transposed [D, S] table are needed.
    cosT = np.ascontiguousarray(cos2.T[:128])
    sinT = np.ascontiguousarray(sin2.T[:128])

    # Additive mask patterns [4, 128, 384]:
    #  0/1/2: diagonal tile at slot 0/1/2 of its chunk (early query tiles)
    #  3:     window leading-edge tile at slot 0 (t >= 8 chunk 0)
    masks4 = np.stack([
        mask[0 * 128:1 * 128, 0:384],        # diag at slot 0, rest -1e9
        mask[4 * 128:5 * 128, 384:768],      # allowed, diag at slot 1, -1e9
        mask[2 * 128:3 * 128, 0:384],        # allowed x2, diag at slot 2
        mask[8 * 128:9 * 128, 0:384],        # leading edge uptri, allowed x2
    ])
    masks4 = round_f32r(masks4)
    ident = round_f32r(np.eye(128, dtype=np.float32))

    in_maps = []
    for core in range(8):
        b, g = core // 4, core % 4
        hsT = round_f32r(np.ascontiguousarray(
            hidden_states[b].T).reshape(KT, 128, S))
        wqT = round_f32r(np.ascontiguousarray(
            Wq[2 * g * D:(2 * g + 2) * D].T).reshape(KT, 128, DQ))
        wkT = round_f32r(np.ascontiguousarray(
            Wk[g * D:(g + 1) * D].T).reshape(KT, 128, D))
        wvT = round_f32r(np.ascontiguousarray(
            Wv[g * D:(g + 1) * D].T).reshape(KT, 128, D))
        woT = round_f32r(np.ascontiguousarray(
            Wo[:, 2 * g * D:(2 * g + 2) * D].T).reshape(NDQ, 128, H))
        qw1p = np.ascontiguousarray(
            (1.0 + np.asarray(q_norm_w, dtype=np.float32)).reshape(NDK, 128).T)
        kw1p = np.ascontiguousarray(
            (1.0 + np.asarray(k_norm_w, dtype=np.float32)).reshape(NDK, 128).T)
        in_maps.append({
            "hsT": hsT, "wqT": wqT, "wkT": wkT, "wvT": wvT, "woT": woT,
            "cosT": cosT, "sinT": sinT, "masks": masks4, "ident": ident,
            "ones_c": np.ones((128, 1), dtype=np.float32),
            "onesr_c": np.ones((1, 128), dtype=np.float32),
            "qw1p": qw1p, "kw1p": kw1p,
        })

    res = run_bass_kernel_spmd(nc, in_maps, core_ids=list(range(8)))
    outs = [r["out"] for r in res.results]
    final = np.zeros((B, S, H), dtype=np.float32)
    for core in range(8):
        b = core // 4
        final[b] += outs[core]
    return final


# revision 12
# speedup vs baseline: 1.3922x; 1.0702x over previous
"""Gemma-style sliding-window attention block on 8 trn2 NeuronCores.

Sharding: tensor-parallel over kv-head groups (4) x data-parallel over
batch (2).  Core c handles batch b = c//4 and kv-head g = c%4 (query
heads 2g, 2g+1).  Each core computes its heads' Q/K/V projections,
RMS norms, RoPE, sliding-window attention and the partial Wo
projection; the host sums the 4 partial outputs per batch.

All matmuls run in float32r (fp32 with 11-bit mantissa, full PE rate at
free-dim >= 256).  Host pre-rounds DMA'd operands; on-chip producers
write f32r directly.

Schedule notes (v2): projections run in 256-seq chunks so the Q/K/V
PSUM banks double-buffer (4 banks/chunk); sum-of-squares and the 1/rms
rank-1 broadcasts use a dedicated single-matmul PSUM stat bank so no
accumulation group shares a bank with a foreign start=True.  Attention
and the output projection are merged per query tile: scores for the
9-tile sliding window land in one [128,3,512] PSUM tile (bufs=2), one
strided exp covers all chunks, attn-weight transposes go 4-at-a-time
through two rotating PSUM banks that double as the Wo accumulators,
and fully-masked chunks of early query tiles are skipped outright.
Masking uses 4 resident additive patterns instead of per-tile DMA.
"""
import numpy as np
from contextlib import ExitStack

import concourse.bass as bass
import concourse.bacc as bacc
import concourse.mybir as mybir
import concourse.tile as tile
from concourse.bass_utils import run_bass_kernel_spmd

F32 = mybir.dt.float32
F32R = mybir.dt.float32r
AL = mybir.AluOpType
AF = mybir.ActivationFunctionType

B, S, H = 2, 2048, 2560
NH, NKV, D = 8, 4, 256
SW = 1024
EPS = 1e-6
ST = S // 128             # 16 sequence tiles
KT = H // 128             # 20 hidden k-tiles
NC_CH = S // 256          # 8 projection chunks of 256
DQ = 2 * D                # per-core query dims (2 heads)
NDQ = DQ // 128           # 4
NDK = D // 128            # 2


def round_f32r(x: np.ndarray) -> np.ndarray:
    """Round fp32 to f32r (11-bit mantissa, round-to-nearest-even)."""
    b = np.ascontiguousarray(x, dtype=np.float32).view(np.uint32).astype(np.uint64)
    bias = 0x7FF + ((b >> 12) & 1)
    return ((b + bias) & 0xFFFFF000).astype(np.uint32).view(np.float32)


def build_nc(debug=False):
    nc = bacc.Bacc("TRN2", target_bir_lowering=False, debug=False)

    hsT_d = nc.dram_tensor("hsT", [KT, 128, S], F32R, kind="ExternalInput")
    wq_d = nc.dram_tensor("wqT", [KT, 128, DQ], F32R, kind="ExternalInput")
    wk_d = nc.dram_tensor("wkT", [KT, 128, D], F32R, kind="ExternalInput")
    wv_d = nc.dram_tensor("wvT", [KT, 128, D], F32R, kind="ExternalInput")
    wo_d = nc.dram_tensor("woT", [NDQ, 128, H], F32R, kind="ExternalInput")
    cos_d = nc.dram_tensor("cosT", [128, S], F32, kind="ExternalInput")
    sin_d = nc.dram_tensor("sinT", [128, S], F32, kind="ExternalInput")
    msk_d = nc.dram_tensor("masks", [4, 128, 384], F32R, kind="ExternalInput")
    idn_d = nc.dram_tensor("ident", [128, 128], F32R, kind="ExternalInput")
    ones_d = nc.dram_tensor("ones_c", [128, 1], F32R, kind="ExternalInput")
    onesr_d = nc.dram_tensor("onesr_c", [1, 128], F32R, kind="ExternalInput")
    qw_d = nc.dram_tensor("qw1p", [128, NDK], F32, kind="ExternalInput")
    kw_d = nc.dram_tensor("kw1p", [128, NDK], F32, kind="ExternalInput")
    out_d = nc.dram_tensor("out", [S, H], F32, kind="ExternalOutput")

    with ExitStack() as top:
        tc = top.enter_context(tile.TileContext(nc))
        big = top.enter_context(tc.tile_pool(name="big", bufs=1))

        # Resident tensors (whole-kernel lifetime)
        QT = [big.tile([128, S], F32R, name=f"QT{j}", tag=f"QT{j}") for j in range(NDQ)]
        KTt = [big.tile([128, S], F32R, name=f"KTt{j}", tag=f"KTt{j}") for j in range(NDK)]
        V = big.tile([128, ST, D], F32R, tag="V")
        masks = big.tile([128, 4, 384], F32R, tag="masks")
        ident = big.tile([128, 128], F32R, tag="ident")
        ones = big.tile([128, 1], F32R, tag="ones")
        onesr = big.tile([1, 128], F32R, tag="onesr")
        epsb = big.tile([128, 1], F32, tag="epsb")
        qw1p = big.tile([128, NDK], F32, tag="qw1p")
        kw1p = big.tile([128, NDK], F32, tag="kw1p")
        nc.vector.memset(epsb, EPS)

        # ---------------- Phase 1: projections + norms + rope -------------
        with ExitStack() as p1:
            wpool = p1.enter_context(tc.tile_pool(name="wpool", bufs=1))
            hpool = p1.enter_context(tc.tile_pool(name="hpool", bufs=5))
            cpool = p1.enter_context(tc.tile_pool(name="cpool", bufs=2))
            sqpool = p1.enter_context(tc.tile_pool(name="sqpool", bufs=2))
            tpool = p1.enter_context(tc.tile_pool(name="tpool", bufs=2))
            spool = p1.enter_context(tc.tile_pool(name="spool", bufs=2))
            pps = p1.enter_context(tc.tile_pool(name="pps", bufs=2, space="PSUM"))
            vpp = p1.enter_context(tc.tile_pool(name="vpp", bufs=1, space="PSUM"))
            stps = p1.enter_context(tc.tile_pool(name="stps", bufs=1, space="PSUM"))

            wq = wpool.tile([128, KT, DQ], F32R, tag="wq")
            wk = wpool.tile([128, KT, D], F32R, tag="wk")
            wv = wpool.tile([128, KT, D], F32R, tag="wv")

            def hst_dma(g, sl):
                h = hpool.tile([128, 5, 256], F32R, name=f"hst{g}", tag="hst")
                nc.sync.dma_start(
                    out=h,
                    in_=hsT_d[g * 5:(g + 1) * 5, :, sl].rearrange("k p s -> p k s"))
                return h

            # DMA issue order is emission order: interleave chunk-0's hidden
            # state with the per-kt weight loads so the PE starts within a
            # few us, instead of draining every weight first.
            sl0 = slice(0, 256)
            hst0 = []
            cos0 = sin0 = None
            for kt in range(KT):
                nc.sync.dma_start(out=wq[:, kt, :], in_=wq_d[kt])
                nc.sync.dma_start(out=wk[:, kt, :], in_=wk_d[kt])
                nc.sync.dma_start(out=wv[:, kt, :], in_=wv_d[kt])
                if kt < 4:
                    hst0.append(hst_dma(kt, sl0))
                elif kt == 4:
                    nc.sync.dma_start(out=ones, in_=ones_d[:, :])
                    nc.sync.dma_start(out=onesr, in_=onesr_d[:, :])
                    nc.sync.dma_start(out=qw1p, in_=qw_d[:, :])
                    nc.sync.dma_start(out=kw1p, in_=kw_d[:, :])
                elif kt == 5:
                    cos0 = cpool.tile([128, 256], F32, tag="cosC")
                    sin0 = cpool.tile([128, 256], F32, tag="sinC")
                    nc.sync.dma_start(out=cos0, in_=cos_d[:, sl0])
                    nc.sync.dma_start(out=sin0, in_=sin_d[:, sl0])
                elif kt == 8:
                    nc.sync.dma_start(out=ident, in_=idn_d[:, :])
                    nc.sync.dma_start(out=masks,
                                      in_=msk_d.rearrange("c p n -> p c n"))

            for sc in range(NC_CH):
                sl = slice(sc * 256, (sc + 1) * 256)
                if sc == 0:
                    hst, cosC, sinC = hst0, cos0, sin0
                else:
                    hst = [hst_dma(g, sl) for g in range(4)]
                    cosC = cpool.tile([128, 256], F32, tag="cosC")
                    sinC = cpool.tile([128, 256], F32, tag="sinC")
                    nc.sync.dma_start(out=cosC, in_=cos_d[:, sl])
                    nc.sync.dma_start(out=sinC, in_=sin_d[:, sl])

                qps = pps.tile([128, NDQ, 256], F32, tag="qps")
                kps = pps.tile([128, NDK, 256], F32, tag="kps")
                vps = vpp.tile([128, 2, 256], F32, tag="vps")
                stat = stps.tile([128, 512], F32, tag="stat")

                for kt in range(KT):
                    h = hst[kt // 5][:, kt % 5, :]
                    st_, sp_ = (kt == 0), (kt == KT - 1)
                    for j in range(NDQ):
                        nc.tensor.matmul(qps[:, j, :], wq[:, kt, j * 128:(j + 1) * 128],
                                         h, start=(st_ and j % 2 == 0), stop=sp_)
                    for j in range(NDK):
                        nc.tensor.matmul(kps[:, j, :], wk[:, kt, j * 128:(j + 1) * 128],
                                         h, start=(st_ and j == 0), stop=sp_)
                    for i in range(2):
                        nc.tensor.matmul(vps[:, i, :], h[:, i * 128:(i + 1) * 128],
                                         wv[:, kt, :], start=(st_ and i == 0), stop=sp_)

                # V rms norm (no weight): rows are sequence positions
                for i in range(2):
                    vscr = tpool.tile([128, 256], F32, tag="vscr")
                    msq = spool.tile([128, 1], F32, tag="msq")
                    nc.scalar.activation(out=vscr, in_=vps[:, i, :],
                                         func=AF.Square, accum_out=msq)
                    sdv = spool.tile([128, 1], F32, tag="sdv")
                    nc.scalar.activation(out=sdv, in_=msq, func=AF.Sqrt,
                                         scale=1.0 / D, bias=epsb)
                    rv = spool.tile([128, 1], F32, tag="rv")
                    nc.vector.reciprocal(out=rv, in_=sdv)
                    nc.vector.tensor_scalar_mul(V[:, sc * 2 + i, :], vps[:, i, :], rv)

                # Q/K rms norm + rope (transposed layout: d on partitions).
                # All stat-bank matmuls are single start/stop groups so a
                # foreign start=True never splits an accumulation pair.
                heads = [(QT, qps, (0, 1), qw1p), (QT, qps, (2, 3), qw1p),
                         (KTt, kps, (0, 1), kw1p)]
                for hidx, (dst, src, (jA, jB), w1p) in enumerate(heads):
                    sq = sqpool.tile([128, 2, 256], F32R, tag="sq")
                    nc.scalar.activation(out=sq, in_=src[:, jA:jA + 2, :],
                                         func=AF.Square)
                    # Sum of squares over both d-tiles: accumulation pair in
                    # the stat bank.  Region overlaps (row 0) serialize each
                    # head's chain, so no foreign start=True can split a pair.
                    ssqw = stat[0:1, 0:256]
                    nc.tensor.matmul(ssqw, ones, sq[:, 0, :], start=True, stop=False)
                    nc.tensor.matmul(ssqw, ones, sq[:, 1, :], start=False, stop=True)
                    sd = spool.tile([1, 256], F32, tag="sd")
                    nc.scalar.activation(out=sd, in_=ssqw, func=AF.Sqrt,
                                         scale=1.0 / D, bias=epsb[0:1, :])
                    rqf = spool.tile([1, 256], F32, tag="rqf")
                    nc.vector.reciprocal(out=rqf, in_=sd)
                    # hi/lo split so the f32r rank-1 broadcast is fp32-exact
                    rq = spool.tile([1, 256], F32R, tag="rq")
                    nc.vector.tensor_copy(out=rq, in_=rqf)
                    rql = spool.tile([1, 256], F32R, tag="rql")
                    with nc.allow_low_precision(reason="f32r lo residual"):
                        nc.vector.tensor_sub(rql, rqf, rq)
                    # bc region contains ssqw's row 0, so the next head's ssq
                    # pair WAR-waits on this head's bc copy: pairs never split.
                    bcps = stat[:, 0:256]
                    nc.tensor.matmul(bcps, onesr, rq, start=True, stop=False)
                    nc.tensor.matmul(bcps, onesr, rql, start=False, stop=True)
                    bc = tpool.tile([128, 256], F32, tag="bc")
                    nc.scalar.copy(out=bc, in_=bcps)
                    qn = []
                    for j, jj in enumerate((jA, jB)):
                        q = tpool.tile([128, 256], F32, name=f"qn{j}", tag=f"qn{j}")
                        nc.vector.scalar_tensor_tensor(
                            out=q, in0=src[:, jj, :], scalar=w1p[:, j:j + 1],
                            in1=bc, op0=AL.mult, op1=AL.mult)
                        qn.append(q)
                    # rope: sin-products and the combines run on GpSimd to
                    # keep the DVE off the critical path (all SBUF operands)
                    t1 = tpool.tile([128, 256], F32, tag="t1")
                    t2 = tpool.tile([128, 256], F32, tag="t2")
                    nc.vector.tensor_mul(t1, qn[0], cosC)
                    nc.gpsimd.tensor_mul(t2, qn[1], sinC)
                    nc.gpsimd.tensor_sub(dst[jA][:, sl], t1, t2)
                    t3 = tpool.tile([128, 256], F32, tag="t1")
                    t4 = tpool.tile([128, 256], F32, tag="t2")
                    nc.vector.tensor_mul(t3, qn[1], cosC)
                    nc.gpsimd.tensor_mul(t4, qn[0], sinC)
                    nc.gpsimd.tensor_add(dst[jB][:, sl], t3, t4)

        # ------- Phase 2+3: attention + output projection, per tile -------
        with ExitStack() as p23:
            wopool = p23.enter_context(tc.tile_pool(name="wopool", bufs=1))
            scp = p23.enter_context(tc.tile_pool(name="scp", bufs=2, space="PSUM"))
            trp_p = p23.enter_context(tc.tile_pool(name="trp_p", bufs=2, space="PSUM"))
            epool = p23.enter_context(tc.tile_pool(name="epool", bufs=2))
            etp = p23.enter_context(tc.tile_pool(name="etp", bufs=2))
            npool = p23.enter_context(tc.tile_pool(name="npool", bufs=4))
            aopool = p23.enter_context(tc.tile_pool(name="aopool", bufs=2))
            opool = p23.enter_context(tc.tile_pool(name="opool", bufs=2))

            woT = wopool.tile([128, NDQ, H], F32R, tag="woT")
            for dj in range(NDQ):
                nc.sync.dma_start(out=woT[:, dj, :], in_=wo_d[dj])

            cp_i = 0  # round-robin PSUM->SBUF copies between DVE and ACT

            def psum_copy(out, in_):
                nonlocal cp_i
                if cp_i % 2 == 0:
                    nc.vector.tensor_copy(out=out, in_=in_)
                else:
                    nc.scalar.copy(out=out, in_=in_)
                cp_i += 1

            for t in range(ST):
                w0 = max(0, t - 8)
                nt = min(t + 1, 9)              # live window tiles
                nch = (nt * 128 + 383) // 384   # live 384-wide score chunks
                aot = aopool.tile([128, NDQ, 128], F32R, tag="aot")
                aot_f = aot.rearrange("p a x -> p (a x)")
                for h in range(2):
                    scs = scp.tile([128, 3, 512], F32, tag="scs")
                    for c in range(nch):
                        rhs_sl = slice(w0 * 128 + c * 384, w0 * 128 + c * 384 + 384)
                        # mask pattern for this chunk (or None)
                        if t < 8:
                            pat = t % 3 if c == t // 3 else None
                        else:
                            pat = 3 if c == 0 else (2 if c == 2 else None)
                        for j in range(NDK):
                            nc.tensor.matmul(
                                scs[:, c, 0:384], QT[2 * h + j][:, t * 128:(t + 1) * 128],
                                KTt[j][:, rhs_sl], start=(j == 0),
                                stop=(j == NDK - 1 and pat is None))
                        if pat is not None:
                            nc.tensor.matmul(scs[:, c, 0:384], ident, masks[:, pat, :],
                                             start=False, stop=True)
                    negm = npool.tile([128, 1], F32, tag="negm")
                    nc.vector.tensor_reduce(out=negm, in_=scs[:, 0:nch, 0:384],
                                            axis=mybir.AxisListType.XY,
                                            op=AL.max, negate=True)
                    expb = epool.tile([128, 3, 384], F32R, tag="expb")
                    expb_f = expb.rearrange("p c x -> p (c x)")
                    dsum = npool.tile([128, 1], F32, tag="dsum")
                    nc.scalar.activation(out=expb[:, 0:nch, :], in_=scs[:, 0:nch, 0:384],
                                         func=AF.Exp, bias=negm, accum_out=dsum)
                    rden = npool.tile([128, 1], F32, tag="rden")
                    nc.vector.reciprocal(out=rden, in_=dsum)
                    expT = etp.tile([128, 9, 128], F32R, tag="expT")
                    expT_f = expT.rearrange("p a x -> p (a x)")
                    for g in range((nt + 3) // 4):
                        trp = trp_p.tile([128, 512], F32R, tag="trp")
                        n_in = min(4, nt - 4 * g)
                        for a2 in range(n_in):
                            a = 4 * g + a2
                            nc.tensor.transpose(
                                trp[:, a2 * 128:(a2 + 1) * 128],
                                expb_f[:, a * 128:(a + 1) * 128], ident)
                        psum_copy(expT_f[:, 4 * g * 128:(4 * g + n_in) * 128],
                                  trp[:, 0:n_in * 128])
                    aop = scs[:, 2, 0:256]
                    for a in range(nt):
                        nc.tensor.matmul(aop, expT[:, a, :], V[:, w0 + a, :],
                                         start=(a == 0), stop=(a == nt - 1))
                    ao = epool.tile([128, 256], F32R, tag="ao")
                    nc.scalar.activation(out=ao, in_=aop, func=AF.Copy, scale=rden)
                    trp2 = trp_p.tile([128, 512], F32R, tag="trp")
                    for j in range(2):
                        nc.tensor.transpose(trp2[:, j * 128:(j + 1) * 128],
                                            ao[:, j * 128:(j + 1) * 128], ident)
                    psum_copy(aot_f[:, 2 * h * 128:(2 * h + 2) * 128],
                              trp2[:, 0:256])

                # ---- output projection for this query tile ----
                osb = opool.tile([128, H], F32, tag="osb")
                for hc in range(H // 512):
                    wop = trp_p.tile([128, 512], F32, name="wop", tag="trp")
                    for dj in range(NDQ):
                        nc.tensor.matmul(
                            wop, aot[:, dj, :], woT[:, dj, hc * 512:(hc + 1) * 512],
                            start=(dj == 0), stop=(dj == NDQ - 1))
                    psum_copy(osb[:, hc * 512:(hc + 1) * 512], wop)
                    if t == ST - 1:
                        # stream the final tile per-chunk so the closing DMA
                        # does not serialize behind the whole-row assembly
                        nc.sync.dma_start(
                            out=out_d[t * 128:(t + 1) * 128,
                                      hc * 512:(hc + 1) * 512],
                            in_=osb[:, hc * 512:(hc + 1) * 512])
                if t < ST - 1:
                    nc.sync.dma_start(
                        out=out_d[t * 128:(t + 1) * 128, :], in_=osb)

    nc.compile()
    return nc


_nc_cache = None


def kernel(hidden_states, attention_mask, cos, sin, Wq, Wk, Wv, Wo,
           q_norm_w, k_norm_w):
    global _nc_cache
    if _nc_cache is None:
        _nc_cache = build_nc()
    nc = _nc_cache

    hidden_states = np.asarray(hidden_states, dtype=np.float32)
    mask = np.asarray(attention_mask, dtype=np.float32)[0, 0]      # [S, S]
    cos2 = np.asarray(cos, dtype=np.float32)[0, 0]                 # [S, D]
    sin2 = np.asarray(sin, dtype=np.float32)[0, 0]
    Wq = np.asarray(Wq, dtype=np.float32)
    Wk = np.asarray(Wk, dtype=np.float32)
    Wv = np.asarray(Wv, dtype=np.float32)
    Wo = np.asarray(Wo, dtype=np.float32)

    # cos/sin have duplicated half-frequencies: only the first 128 rows of
    # the transposed [D, S] table are needed.
    cosT = np.ascontiguousarray(cos2.T[:128])
    sinT = np.ascontiguousarray(sin2.T[:128])

    # Additive mask patterns [4, 128, 384]:
    #  0/1/2: diagonal tile at slot 0/1/2 of its chunk (early query tiles)
    #  3:     window leading-edge tile at slot 0 (t >= 8 chunk 0)
    masks4 = np.stack([
        mask[0 * 128:1 * 128, 0:384],        # diag at slot 0, rest -1e9
        mask[4 * 128:5 * 128, 384:768],      # allowed, diag at slot 1, -1e9
        mask[2 * 128:3 * 128, 0:384],        # allowed x2, diag at slot 2
        mask[8 * 128:9 * 128, 0:384],        # leading edge uptri, allowed x2
    ])
    masks4 = round_f32r(masks4)
    ident = round_f32r(np.eye(128, dtype=np.float32))

    in_maps = []
    for core in range(8):
        b, g = core // 4, core % 4
        hsT = round_f32r(np.ascontiguousarray(
            hidden_states[b].T).reshape(KT, 128, S))
        wqT = round_f32r(np.ascontiguousarray(
            Wq[2 * g * D:(2 * g + 2) * D].T).reshape(KT, 128, DQ))
        wkT = round_f32r(np.ascontiguousarray(
            Wk[g * D:(g + 1) * D].T).reshape(KT, 128, D))
        wvT = round_f32r(np.ascontiguousarray(
            Wv[g * D:(g + 1) * D].T).reshape(KT, 128, D))
        woT = round_f32r(np.ascontiguousarray(
            Wo[:, 2 * g * D:(2 * g + 2) * D].T).reshape(NDQ, 128, H))
        qw1p = np.ascontiguousarray(
            (1.0 + np.asarray(q_norm_w, dtype=np.float32)).reshape(NDK, 128).T)
        kw1p = np.ascontiguousarray(
            (1.0 + np.asarray(k_norm_w, dtype=np.float32)).reshape(NDK, 128).T)
        in_maps.append({
            "hsT": hsT, "wqT": wqT, "wkT": wkT, "wvT": wvT, "woT": woT,
            "cosT": cosT, "sinT": sinT, "masks": masks4, "ident": ident,
            "ones_c": np.ones((128, 1), dtype=np.float32),
            "onesr_c": np.ones((1, 128), dtype=np.float32),
            "qw1p": qw1p, "kw1p": kw1p,
        })

    res = run_bass_kernel_spmd(nc, in_maps, core_ids=list(range(8)))
    outs = [r["out"] for r in res.results]
    final = np.zeros((B, S, H), dtype=np.float32)
    for core in range(8):
        b = core // 4
        final[b] += outs[core]
    return final


# revision 15
# speedup vs baseline: 1.4387x; 1.0334x over previous
"""Gemma-style sliding-window attention block on 8 trn2 NeuronCores.

Sharding: tensor-parallel over kv-head groups (4) x data-parallel over
batch (2).  Core c handles batch b = c//4 and kv-head g = c%4 (query
heads 2g, 2g+1).  Each core computes its heads' Q/K/V projections,
RMS norms, RoPE, sliding-window attention and the partial Wo
projection; the host sums the 4 partial outputs per batch.

All matmuls run in float32r (fp32 with 11-bit mantissa, full PE rate at
free-dim >= 256).  Host pre-rounds DMA'd operands; on-chip producers
write f32r directly.

Schedule notes (v2): projections run in 256-seq chunks so the Q/K/V
PSUM banks double-buffer (4 banks/chunk); sum-of-squares and the 1/rms
rank-1 broadcasts use a dedicated single-matmul PSUM stat bank so no
accumulation group shares a bank with a foreign start=True.  Attention
and the output projection are merged per query tile: scores for the
9-tile sliding window land in one [128,3,512] PSUM tile (bufs=2), one
strided exp covers all chunks, attn-weight transposes go 4-at-a-time
through two rotating PSUM banks that double as the Wo accumulators,
and fully-masked chunks of early query tiles are skipped outright.
Masking uses 4 resident additive patterns instead of per-tile DMA.
"""
import numpy as np
from contextlib import ExitStack

import concourse.bass as bass
import concourse.bacc as bacc
import concourse.mybir as mybir
import concourse.tile as tile
from concourse.bass_utils import run_bass_kernel_spmd

F32 = mybir.dt.float32
F32R = mybir.dt.float32r
AL = mybir.AluOpType
AF = mybir.ActivationFunctionType

B, S, H = 2, 2048, 2560
NH, NKV, D = 8, 4, 256
SW = 1024
EPS = 1e-6
ST = S // 128             # 16 sequence tiles
KT = H // 128             # 20 hidden k-tiles
NC_CH = S // 256          # 8 projection chunks of 256
DQ = 2 * D                # per-core query dims (2 heads)
NDQ = DQ // 128           # 4
NDK = D // 128            # 2


def round_f32r(x: np.ndarray) -> np.ndarray:
    """Round fp32 to f32r (11-bit mantissa, round-to-nearest-even)."""
    b = np.ascontiguousarray(x, dtype=np.float32).view(np.uint32).astype(np.uint64)
    bias = 0x7FF + ((b >> 12) & 1)
    return ((b + bias) & 0xFFFFF000).astype(np.uint32).view(np.float32)


def build_nc(debug=False):
    nc = bacc.Bacc("TRN2", target_bir_lowering=False, debug=False)

    hsT_d = nc.dram_tensor("hsT", [KT, 128, S], F32R, kind="ExternalInput")
    wq_d = nc.dram_tensor("wqT", [KT, 128, DQ], F32R, kind="ExternalInput")
    wk_d = nc.dram_tensor("wkT", [KT, 128, D], F32R, kind="ExternalInput")
    wv_d = nc.dram_tensor("wvT", [KT, 128, D], F32R, kind="ExternalInput")
    wo_d = nc.dram_tensor("woT", [NDQ, 128, H], F32R, kind="ExternalInput")
    cos_d = nc.dram_tensor("cosT", [128, S], F32, kind="ExternalInput")
    sin_d = nc.dram_tensor("sinT", [128, S], F32, kind="ExternalInput")
    msk_d = nc.dram_tensor("masks", [4, 128, 384], F32R, kind="ExternalInput")
    idn_d = nc.dram_tensor("ident", [128, 128], F32R, kind="ExternalInput")
    ones_d = nc.dram_tensor("ones_c", [128, 1], F32R, kind="ExternalInput")
    qw_d = nc.dram_tensor("qw1p", [128, NDK], F32, kind="ExternalInput")
    kw_d = nc.dram_tensor("kw1p", [128, NDK], F32, kind="ExternalInput")
    out_d = nc.dram_tensor("out", [S, H], F32, kind="ExternalOutput")

    with ExitStack() as top:
        tc = top.enter_context(tile.TileContext(nc))
        big = top.enter_context(tc.tile_pool(name="big", bufs=1))

        # Resident tensors (whole-kernel lifetime)
        QT = [big.tile([128, S], F32R, name=f"QT{j}", tag=f"QT{j}") for j in range(NDQ)]
        KTt = [big.tile([128, S], F32R, name=f"KTt{j}", tag=f"KTt{j}") for j in range(NDK)]
        V = big.tile([128, ST, D], F32R, tag="V")
        masks = big.tile([128, 4, 384], F32R, tag="masks")
        ident = big.tile([128, 128], F32R, tag="ident")
        ones = big.tile([128, 1], F32R, tag="ones")
        epsb = big.tile([128, 1], F32, tag="epsb")
        qw1p = big.tile([128, NDK], F32, tag="qw1p")
        kw1p = big.tile([128, NDK], F32, tag="kw1p")
        nc.vector.memset(epsb, EPS)

        # ---------------- Phase 1: projections + norms + rope -------------
        with ExitStack() as p1:
            wpool = p1.enter_context(tc.tile_pool(name="wpool", bufs=1))
            hpool = p1.enter_context(tc.tile_pool(name="hpool", bufs=5))
            cpool = p1.enter_context(tc.tile_pool(name="cpool", bufs=2))
            sqpool = p1.enter_context(tc.tile_pool(name="sqpool", bufs=2))
            tpool = p1.enter_context(tc.tile_pool(name="tpool", bufs=2))
            spool = p1.enter_context(tc.tile_pool(name="spool", bufs=2))
            pps = p1.enter_context(tc.tile_pool(name="pps", bufs=2, space="PSUM"))
            vpp = p1.enter_context(tc.tile_pool(name="vpp", bufs=1, space="PSUM"))
            stps = p1.enter_context(tc.tile_pool(name="stps", bufs=1, space="PSUM"))

            wq = wpool.tile([128, KT, DQ], F32R, tag="wq")
            wk = wpool.tile([128, KT, D], F32R, tag="wk")
            wv = wpool.tile([128, KT, D], F32R, tag="wv")

            def hst_dma(g, sl):
                h = hpool.tile([128, 5, 256], F32R, name=f"hst{g}", tag="hst")
                nc.sync.dma_start(
                    out=h,
                    in_=hsT_d[g * 5:(g + 1) * 5, :, sl].rearrange("k p s -> p k s"))
                return h

            # DMA issue order is emission order: interleave chunk-0's hidden
            # state with grouped weight loads (HWDGE costs ~625ns per DMA,
            # so fewer/bigger weight transfers keep the queue short).
            sl0 = slice(0, 256)
            hst0 = []
            cos0 = sin0 = None
            for w5 in range(5):
                k0 = 4 * w5
                for wt, wt_d in ((wq, wq_d), (wk, wk_d), (wv, wv_d)):
                    nc.sync.dma_start(
                        out=wt[:, k0:k0 + 4, :],
                        in_=wt_d[k0:k0 + 4].rearrange("k p m -> p k m"))
                if w5 < 4:
                    hst0.append(hst_dma(w5, sl0))
                if w5 == 0:
                    nc.sync.dma_start(out=ones, in_=ones_d[:, :])
                    nc.sync.dma_start(out=qw1p, in_=qw_d[:, :])
                    nc.sync.dma_start(out=kw1p, in_=kw_d[:, :])
                elif w5 == 2:
                    cos0 = cpool.tile([128, 256], F32, tag="cosC")
                    sin0 = cpool.tile([128, 256], F32, tag="sinC")
                    nc.sync.dma_start(out=cos0, in_=cos_d[:, sl0])
                    nc.sync.dma_start(out=sin0, in_=sin_d[:, sl0])
                elif w5 == 3:
                    nc.sync.dma_start(out=ident, in_=idn_d[:, :])
                    nc.sync.dma_start(out=masks,
                                      in_=msk_d.rearrange("c p n -> p c n"))

            for sc in range(NC_CH):
                sl = slice(sc * 256, (sc + 1) * 256)
                if sc == 0:
                    hst, cosC, sinC = hst0, cos0, sin0
                else:
                    hst = [hst_dma(g, sl) for g in range(4)]
                    cosC = cpool.tile([128, 256], F32, tag="cosC")
                    sinC = cpool.tile([128, 256], F32, tag="sinC")
                    nc.sync.dma_start(out=cosC, in_=cos_d[:, sl])
                    nc.sync.dma_start(out=sinC, in_=sin_d[:, sl])

                qps = pps.tile([128, NDQ, 256], F32, tag="qps")
                kps = pps.tile([128, NDK, 256], F32, tag="kps")
                vps = vpp.tile([128, 2, 256], F32, tag="vps")
                stat = stps.tile([128, 512], F32, tag="stat")

                for kt in range(KT):
                    h = hst[kt // 5][:, kt % 5, :]
                    st_, sp_ = (kt == 0), (kt == KT - 1)
                    for j in range(NDQ):
                        nc.tensor.matmul(qps[:, j, :], wq[:, kt, j * 128:(j + 1) * 128],
                                         h, start=(st_ and j % 2 == 0), stop=sp_)
                    for j in range(NDK):
                        nc.tensor.matmul(kps[:, j, :], wk[:, kt, j * 128:(j + 1) * 128],
                                         h, start=(st_ and j == 0), stop=sp_)
                    for i in range(2):
                        nc.tensor.matmul(vps[:, i, :], h[:, i * 128:(i + 1) * 128],
                                         wv[:, kt, :], start=(st_ and i == 0), stop=sp_)

                # V rms norm (no weight): rows are sequence positions
                for i in range(2):
                    vscr = tpool.tile([128, 256], F32, tag="vscr")
                    msq = spool.tile([128, 1], F32, tag="msq")
                    nc.scalar.activation(out=vscr, in_=vps[:, i, :],
                                         func=AF.Square, accum_out=msq)
                    sdv = spool.tile([128, 1], F32, tag="sdv")
                    nc.scalar.activation(out=sdv, in_=msq, func=AF.Sqrt,
                                         scale=1.0 / D, bias=epsb)
                    rv = spool.tile([128, 1], F32, tag="rv")
                    nc.vector.reciprocal(out=rv, in_=sdv)
                    nc.vector.tensor_scalar_mul(V[:, sc * 2 + i, :], vps[:, i, :], rv)

                # Q/K rms norm + rope (transposed layout: d on partitions).
                # All stat-bank matmuls are single start/stop groups so a
                # foreign start=True never splits an accumulation pair.
                heads = [(QT, qps, (0, 1), qw1p), (QT, qps, (2, 3), qw1p),
                         (KTt, kps, (0, 1), kw1p)]
                for hidx, (dst, src, (jA, jB), w1p) in enumerate(heads):
                    sq = sqpool.tile([128, 2, 256], F32R, tag="sq")
                    nc.scalar.activation(out=sq, in_=src[:, jA:jA + 2, :],
                                         func=AF.Square)
                    # Sum of squares over both d-tiles: accumulation pair in
                    # the stat bank.  Region overlaps (row 0) serialize each
                    # head's chain, so no foreign start=True can split a pair.
                    ssqw = stat[0:1, 0:256]
                    nc.tensor.matmul(ssqw, ones, sq[:, 0, :], start=True, stop=False)
                    nc.tensor.matmul(ssqw, ones, sq[:, 1, :], start=False, stop=True)
                    sd = spool.tile([1, 256], F32, tag="sd")
                    nc.scalar.activation(out=sd, in_=ssqw, func=AF.Sqrt,
                                         scale=1.0 / D, bias=epsb[0:1, :])
                    rqf = spool.tile([1, 256], F32, tag="rqf")
                    nc.vector.reciprocal(out=rqf, in_=sd)
                    # exact fp32 broadcast of 1/rms across partitions; also
                    # keeps the stat bank free of a second accumulation group
                    bc = tpool.tile([128, 256], F32, tag="bc")
                    nc.gpsimd.partition_broadcast(bc, rqf)
                    qn = []
                    for j, jj in enumerate((jA, jB)):
                        q = tpool.tile([128, 256], F32, name=f"qn{j}", tag=f"qn{j}")
                        nc.vector.scalar_tensor_tensor(
                            out=q, in0=src[:, jj, :], scalar=w1p[:, j:j + 1],
                            in1=bc, op0=AL.mult, op1=AL.mult)
                        qn.append(q)
                    # rope: sin-products and the combines run on GpSimd to
                    # keep the DVE off the critical path (all SBUF operands)
                    t1 = tpool.tile([128, 256], F32, tag="t1")
                    t2 = tpool.tile([128, 256], F32, tag="t2")
                    nc.vector.tensor_mul(t1, qn[0], cosC)
                    nc.gpsimd.tensor_mul(t2, qn[1], sinC)
                    nc.gpsimd.tensor_sub(dst[jA][:, sl], t1, t2)
                    t3 = tpool.tile([128, 256], F32, tag="t1")
                    t4 = tpool.tile([128, 256], F32, tag="t2")
                    nc.vector.tensor_mul(t3, qn[1], cosC)
                    nc.gpsimd.tensor_mul(t4, qn[0], sinC)
                    nc.gpsimd.tensor_add(dst[jB][:, sl], t3, t4)

        # ------- Phase 2+3: attention + output projection, per tile -------
        with ExitStack() as p23:
            wopool = p23.enter_context(tc.tile_pool(name="wopool", bufs=1))
            scp = p23.enter_context(tc.tile_pool(name="scp", bufs=2, space="PSUM"))
            trp_p = p23.enter_context(tc.tile_pool(name="trp_p", bufs=2, space="PSUM"))
            epool = p23.enter_context(tc.tile_pool(name="epool", bufs=2))
            etp = p23.enter_context(tc.tile_pool(name="etp", bufs=2))
            npool = p23.enter_context(tc.tile_pool(name="npool", bufs=4))
            aopool = p23.enter_context(tc.tile_pool(name="aopool", bufs=2))
            opool = p23.enter_context(tc.tile_pool(name="opool", bufs=2))

            woT = wopool.tile([128, NDQ, H], F32R, tag="woT")
            for dj in range(NDQ):
                nc.sync.dma_start(out=woT[:, dj, :], in_=wo_d[dj])

            cp_i = 0  # round-robin PSUM->SBUF copies between DVE and ACT

            def psum_copy(out, in_):
                nonlocal cp_i
                if cp_i % 2 == 0:
                    nc.vector.tensor_copy(out=out, in_=in_)
                else:
                    nc.scalar.copy(out=out, in_=in_)
                cp_i += 1

            for t in range(ST):
                w0 = max(0, t - 8)
                nt = min(t + 1, 9)              # live window tiles
                nch = (nt * 128 + 383) // 384   # live 384-wide score chunks
                aot = aopool.tile([128, NDQ, 128], F32R, tag="aot")
                aot_f = aot.rearrange("p a x -> p (a x)")
                for h in range(2):
                    scs = scp.tile([128, 3, 512], F32, tag="scs")
                    for c in range(nch):
                        rhs_sl = slice(w0 * 128 + c * 384, w0 * 128 + c * 384 + 384)
                        # mask pattern for this chunk (or None)
                        if t < 8:
                            pat = t % 3 if c == t // 3 else None
                        else:
                            pat = 3 if c == 0 else (2 if c == 2 else None)
                        for j in range(NDK):
                            nc.tensor.matmul(
                                scs[:, c, 0:384], QT[2 * h + j][:, t * 128:(t + 1) * 128],
                                KTt[j][:, rhs_sl], start=(j == 0),
                                stop=(j == NDK - 1 and pat is None))
                        if pat is not None:
                            nc.tensor.matmul(scs[:, c, 0:384], ident, masks[:, pat, :],
                                             start=False, stop=True)
                    negm = npool.tile([128, 1], F32, tag="negm")
                    nc.vector.tensor_reduce(out=negm, in_=scs[:, 0:nch, 0:384],
                                            axis=mybir.AxisListType.XY,
                                            op=AL.max, negate=True)
                    expb = epool.tile([128, 3, 384], F32R, tag="expb")
                    expb_f = expb.rearrange("p c x -> p (c x)")
                    dsum = npool.tile([128, 1], F32, tag="dsum")
                    nc.scalar.activation(out=expb[:, 0:nch, :], in_=scs[:, 0:nch, 0:384],
                                         func=AF.Exp, bias=negm, accum_out=dsum)
                    rden = npool.tile([128, 1], F32, tag="rden")
                    nc.vector.reciprocal(out=rden, in_=dsum)
                    expT = etp.tile([128, 9, 128], F32R, tag="expT")
                    expT_f = expT.rearrange("p a x -> p (a x)")
                    for g in range((nt + 3) // 4):
                        trp = trp_p.tile([128, 512], F32R, tag="trp")
                        n_in = min(4, nt - 4 * g)
                        for a2 in range(n_in):
                            a = 4 * g + a2
                            nc.tensor.transpose(
                                trp[:, a2 * 128:(a2 + 1) * 128],
                                expb_f[:, a * 128:(a + 1) * 128], ident)
                        psum_copy(expT_f[:, 4 * g * 128:(4 * g + n_in) * 128],
                                  trp[:, 0:n_in * 128])
                    aop = scs[:, 2, 0:256]
                    for a in range(nt):
                        nc.tensor.matmul(aop, expT[:, a, :], V[:, w0 + a, :],
                                         start=(a == 0), stop=(a == nt - 1))
                    ao = epool.tile([128, 256], F32R, tag="ao")
                    nc.scalar.activation(out=ao, in_=aop, func=AF.Copy, scale=rden)
                    trp2 = trp_p.tile([128, 512], F32R, tag="trp")
                    for j in range(2):
                        nc.tensor.transpose(trp2[:, j * 128:(j + 1) * 128],
                                            ao[:, j * 128:(j + 1) * 128], ident)
                    psum_copy(aot_f[:, 2 * h * 128:(2 * h + 2) * 128],
                              trp2[:, 0:256])

                # ---- output projection for this query tile ----
                osb = opool.tile([128, H], F32, tag="osb")
                for hc in range(H // 512):
                    wop = trp_p.tile([128, 512], F32, name="wop", tag="trp")
                    for dj in range(NDQ):
                        nc.tensor.matmul(
                            wop, aot[:, dj, :], woT[:, dj, hc * 512:(hc + 1) * 512],
                            start=(dj == 0), stop=(dj == NDQ - 1))
                    psum_copy(osb[:, hc * 512:(hc + 1) * 512], wop)
                    if t == ST - 1:
                        # stream the final tile per-chunk so the closing DMA
                        # does not serialize behind the whole-row assembly
                        nc.sync.dma_start(
                            out=out_d[t * 128:(t + 1) * 128,
                                      hc * 512:(hc + 1) * 512],
                            in_=osb[:, hc * 512:(hc + 1) * 512])
                if t < ST - 1:
                    nc.sync.dma_start(
                        out=out_d[t * 128:(t + 1) * 128, :], in_=osb)

    nc.compile()
    return nc


_nc_cache = None


def kernel(hidden_states, attention_mask, cos, sin, Wq, Wk, Wv, Wo,
           q_norm_w, k_norm_w):
    global _nc_cache
    if _nc_cache is None:
        _nc_cache = build_nc()
    nc = _nc_cache

    hidden_states = np.asarray(hidden_states, dtype=np.float32)
    mask = np.asarray(attention_mask, dtype=np.float32)[0, 0]      # [S, S]
    cos2 = np.asarray(cos, dtype=np.float32)[0, 0]                 # [S, D]
    sin2 = np.asarray(sin, dtype=np.float32)[0, 0]
    Wq = np.asarray(Wq, dtype=np.float32)
    Wk = np.asarray(Wk, dtype=np.float32)
    Wv = np.asarray(Wv, dtype=np.float32)
    Wo = np.asarray(Wo, dtype=np.float32)

    # cos/sin have duplicated half-frequencies: only the first 128 rows of
    # the transposed [D, S] table are needed.
    cosT = np.ascontiguousarray(cos2.T[:128])
    sinT = np.ascontiguousarray(sin2.T[:128])

    # Additive mask patterns [4, 128, 384]:
    #  0/1/2: diagonal tile at slot 0/1/2 of its chunk (early query tiles)
    #  3:     window leading-edge tile at slot 0 (t >= 8 chunk 0)
    masks4 = np.stack([
        mask[0 * 128:1 * 128, 0:384],        # diag at slot 0, rest -1e9
        mask[4 * 128:5 * 128, 384:768],      # allowed, diag at slot 1, -1e9
        mask[2 * 128:3 * 128, 0:384],        # allowed x2, diag at slot 2
        mask[8 * 128:9 * 128, 0:384],        # leading edge uptri, allowed x2
    ])
    masks4 = round_f32r(masks4)
    ident = round_f32r(np.eye(128, dtype=np.float32))

    in_maps = []
    for core in range(8):
        b, g = core // 4, core % 4
        hsT = round_f32r(np.ascontiguousarray(
            hidden_states[b].T).reshape(KT, 128, S))
        wqT = round_f32r(np.ascontiguousarray(
            Wq[2 * g * D:(2 * g + 2) * D].T).reshape(KT, 128, DQ))
        wkT = round_f32r(np.ascontiguousarray(
            Wk[g * D:(g + 1) * D].T).reshape(KT, 128, D))
        wvT = round_f32r(np.ascontiguousarray(
            Wv[g * D:(g + 1) * D].T).reshape(KT, 128, D))
        woT = round_f32r(np.ascontiguousarray(
            Wo[:, 2 * g * D:(2 * g + 2) * D].T).reshape(NDQ, 128, H))
        qw1p = np.ascontiguousarray(
            (1.0 + np.asarray(q_norm_w, dtype=np.float32)).reshape(NDK, 128).T)
        kw1p = np.ascontiguousarray(
            (1.0 + np.asarray(k_norm_w, dtype=np.float32)).reshape(NDK, 128).T)
        in_maps.append({
            "hsT": hsT, "wqT": wqT, "wkT": wkT, "wvT": wvT, "woT": woT,
            "cosT": cosT, "sinT": sinT, "masks": masks4, "ident": ident,
            "ones_c": np.ones((128, 1), dtype=np.float32),
            "qw1p": qw1p, "kw1p": kw1p,
        })

    res = run_bass_kernel_spmd(nc, in_maps, core_ids=list(range(8)))
    outs = [r["out"] for r in res.results]
    final = np.zeros((B, S, H), dtype=np.float32)
    for core in range(8):
        b = core // 4
        final[b] += outs[core]
    return final
